# revision 21
# baseline (speedup 1.0000x reference)
"""Trainium2 Bass kernel for the exp-kernel multivariate Hawkes process
log-likelihood (B=8, N=2048, D=10).

v2 strategy (rewrite of the chunked O(N*D^2) baseline)
------------------------------------------------------
Data-parallel over batch: core b computes batch row b. Chunks of C=128
events; per chunk the pairwise interaction is a PE prefix-matmul over
(r,m) type pairs; inter-chunk state S is an affine scan chained PER
GROUP of 4 chunks so the whole S machinery pipelines with the W build
instead of serializing at the end.

Key deltas vs v1:
- maskab host tensor (ab[r,m]*[e_i==r], bf16) folds the receiver mask
  into Q = expU*maskab, eliminating the PSUM-masking pass.
- Per-group scan with `initial` chaining + diag-placement inject
  (K=100 matmul) replaces transpose/selmask/K=16 injects.
- Negative part gathers (bT | ln aT) rows via 2 block-diagonal K=80
  matmuls and fuses the alpha weight into the exp argument.
- Device output is raw (lamr | negsub) [128, 32]; the host does the
  final log/sum reduction in fp64 (O(N) numpy), so no Ln table load
  and no device-side final reduce.
- Inputs consolidated into 4 packed DRAM tensors DMA'd on parallel
  queues immediately at kernel start.
"""
import numpy as np
from contextlib import ExitStack

import ml_dtypes
import concourse.bass as bass
import concourse.mybir as mybir
import concourse.tile as tile
from concourse import bacc
from concourse.bass_utils import run_bass_kernel_spmd

f32 = mybir.dt.float32
bf16 = mybir.dt.bfloat16
AL = mybir.AluOpType
AF = mybir.ActivationFunctionType
AX = mybir.AxisListType

P = 128          # partitions == chunk size
KC = 16          # number of chunks
D = 10           # event types
RM = D * D       # (receiver, trigger) pairs
N = P * KC       # 2048 events per batch row
B = 8            # batch == cores
NG = 4           # chunk groups (4 chunks per PSUM bank)
GC = KC // NG    # chunks per group

# packed DRAM inputs: name -> (shape, dtype)
INPUTS = {
    # trel(16) tau2(16) bflat(100) decayT(16, partitions 0:100)
    "a32": ((P, 148), f32),
    # maskab (c,r,m) flat = 1600
    "mka": ((P, KC * RM), bf16),
    # ohM (c,m) 0:160 | triu 160:288 | diagsel 288:388 (partitions 0:100)
    "msc": ((P, 388), bf16),
    # grows lhsT halves 0:128, 128:256 | block-diag tabs 256:416
    "gro": ((80, 416), bf16),
}


def _body(ctx: ExitStack, tc, ins, out_ap):
    nc = tc.nc
    cpool = ctx.enter_context(tc.tile_pool(name="cpool", bufs=1))
    wpool = ctx.enter_context(tc.tile_pool(name="wpool", bufs=1))
    pp = ctx.enter_context(tc.tile_pool(name="pp", bufs=1, space="PSUM"))
    ps = ctx.enter_context(tc.tile_pool(name="ps", bufs=1, space="PSUM"))

    # ---- input DMAs, all issued first on parallel queues ----
    a32 = cpool.tile([P, 148], f32, tag="a32")
    nc.sync.dma_start(out=a32[:], in_=ins["a32"])
    mka = cpool.tile([P, KC, RM], bf16, tag="mka")
    nc.scalar.dma_start(out=mka[:, 0:KC // 2], in_=ins["mka"][:, 0:800])
    nc.gpsimd.dma_start(out=mka[:, KC // 2:], in_=ins["mka"][:, 800:1600])
    msc = cpool.tile([P, 388], bf16, tag="msc")
    nc.scalar.dma_start(out=msc[:], in_=ins["msc"])
    gro = cpool.tile([80, 416], bf16, tag="gro")
    nc.gpsimd.dma_start(out=gro[:], in_=ins["gro"])

    trel = a32[:, 0:16]
    tau2 = a32[:, 16:32]
    bflat = a32[:, 32:132]
    decayT = a32[0:RM, 132:148]
    ohM = msc[:, 0:160].rearrange("p (c m) -> p c m", c=KC)
    triu = msc[:, 160:288]
    diagsel = msc[0:RM, 288:388]

    # ---- constants ----
    ones_col_bf = cpool.tile([P, 1], bf16, tag="ones_col_bf")
    nc.vector.memset(ones_col_bf[:], 1.0)
    ones100 = cpool.tile([RM, P], bf16, tag="ones100")
    nc.vector.memset(ones100[:], 1.0)
    # SCOLbuf[:, k] = S_k (col 0 = S_0 = 0); scans chain through it.
    # bf16: scan state stays fp32 internally, only stores downcast.
    SCOL = cpool.tile([RM, KC + 1], bf16, tag="SCOL")
    nc.vector.memset(SCOL[:, 0:1], 0.0)

    # ---- big per-event tiles ----
    argW = wpool.tile([P, KC, D, D], f32, tag="argW")
    expW = wpool.tile([P, KC, D, D], bf16, tag="expW")
    expU = wpool.tile([P, KC, D, D], bf16, tag="expU")
    W = wpool.tile([P, KC, D, D], bf16, tag="W")
    Q = wpool.tile([P, KC, RM], bf16, tag="Q")
    G2 = wpool.tile([P, KC, RM], bf16, tag="G2")
    rhsd = wpool.tile([RM, KC, RM], bf16, tag="rhsd")
    outt = wpool.tile([P, 32], f32, tag="outt")

    # asymmetric chunk groups: big early groups pipeline, tiny last group
    # keeps the end-of-kernel serial chain (scan->rhsd->inject->G2->lamr)
    # short
    GROUPS = [(0, 5), (5, 10), (10, 15), (15, 16)]
    Pg = [pp.tile([P, hi - lo, D, D], f32, tag=f"Pg{g}", name=f"Pg{g}")
          for g, (lo, hi) in enumerate(GROUPS)]
    wsum = ps.tile([RM, KC], f32, tag="wsum")

    grows = ps.tile([P, 2, 8, 20], f32, tag="grows")
    grosb = wpool.tile([P, 2, 8, 20], f32, tag="grosb")
    narg = wpool.tile([P, 2, 8, D], f32, tag="narg")
    nexp = wpool.tile([P, 2, 8, D], bf16, tag="nexp")
    NGR = len(GROUPS)

    # Issue order is phase-separated per engine: early-runnable ops go
    # first in each queue so long-stalling ops (scan/G2/lamr, which wait on
    # PE results) don't fill the 4-deep wait queue and block dispatch.

    # -- phase 1: args + exps + masks --
    for g, (lo, hi) in enumerate(GROUPS):
        gs, gc = slice(lo, hi), hi - lo
        nc.vector.tensor_tensor(
            out=argW[:, gs],
            in0=trel[:, gs].unsqueeze(2).unsqueeze(3)
                .broadcast_to([P, gc, D, D]),
            in1=bflat.rearrange("p (r m) -> p r m", r=D)
                .unsqueeze(1).broadcast_to([P, gc, D, D]),
            op=AL.mult)
        nc.scalar.activation(expW[:, gs], argW[:, gs], AF.Exp)
        nc.scalar.activation(expU[:, gs], argW[:, gs], AF.Exp, scale=-1.0)
        nc.gpsimd.tensor_tensor(
            out=W[:, gs], in0=expW[:, gs],
            in1=ohM[:, gs].unsqueeze(2).broadcast_to([P, gc, D, D]),
            op=AL.mult)
    for g, (lo, hi) in enumerate(GROUPS):
        gs = slice(lo, hi)
        nc.vector.tensor_tensor(
            out=Q[:, gs],
            in0=expU[:, gs].rearrange("p c r m -> p c (r m)"),
            in1=mka[:, gs], op=AL.mult)

    # -- phase 2: PE prefix + column sums; gathers for the integral part --
    for h in range(2):
        nc.tensor.matmul(grows[:, h].rearrange("p c t -> p (c t)"),
                         gro[:, 128 * h:128 * (h + 1)],
                         gro[:, 256:416], start=True, stop=True)
    for g, (lo, hi) in enumerate(GROUPS):
        gs = slice(lo, hi)
        for k in range(lo, hi):
            nc.tensor.matmul(wsum[:, k:k + 1],
                             W[:, k].rearrange("p r m -> p (r m)"),
                             ones_col_bf[:], start=True, stop=True)
        nc.tensor.matmul(Pg[g][:], triu,
                         W[:, gs].rearrange("p c r m -> p (c r m)"),
                         start=True, stop=False)

    # -- phase 3: scans (DVE) + diag rows (Pool; DVE for last) + injects --
    for g, (lo, hi) in enumerate(GROUPS):
        gs, gc = slice(lo, hi), hi - lo
        nc.vector.tensor_tensor_scan(
            SCOL[:, lo + 1:hi + 1], wsum[:, gs],
            decayT[:, gs], initial=SCOL[:, lo:lo + 1],
            op0=AL.add, op1=AL.mult)
        rhsd_eng = nc.vector if g == NGR - 1 else nc.gpsimd
        rhsd_eng.tensor_tensor(
            out=rhsd[:, gs],
            in0=SCOL[:, gs].unsqueeze(2).broadcast_to([RM, gc, RM]),
            in1=diagsel.unsqueeze(1).broadcast_to([RM, gc, RM]),
            op=AL.mult)
        nc.tensor.matmul(Pg[g][:], ones100[:],
                         rhsd[:, gs].rearrange("k c rm -> k (c rm)"),
                         start=False, stop=True)

    # -- negative (integral) part: Pool + scalar, reduces queued on DVE
    # before the G2 tail so they run in the inject shadow --
    for h in range(2):
        nc.scalar.activation(grosb[:, h], grows[:, h], AF.Copy)
        nc.gpsimd.tensor_tensor(
            out=narg[:, h], in0=grosb[:, h, :, 0:10],
            in1=tau2[:, 8 * h:8 * (h + 1)].unsqueeze(2)
                .broadcast_to([P, 8, D]),
            op=AL.mult)
        nc.gpsimd.tensor_tensor(
            out=narg[:, h], in0=narg[:, h], in1=grosb[:, h, :, 10:20],
            op=AL.add)
        nc.scalar.activation(nexp[:, h], narg[:, h], AF.Exp)
        nc.vector.tensor_reduce(
            out=outt[:, 16 + 8 * h:16 + 8 * (h + 1)], in_=nexp[:, h],
            axis=AX.X, op=AL.add)

    # -- phase 4: lam tail --
    for g, (lo, hi) in enumerate(GROUPS):
        gs = slice(lo, hi)
        nc.vector.tensor_tensor(
            out=G2[:, gs],
            in0=Pg[g][:].rearrange("p c r m -> p c (r m)"),
            in1=Q[:, gs], op=AL.mult)
        nc.vector.tensor_reduce(
            out=outt[:, lo:hi], in_=G2[:, gs],
            axis=AX.X, op=AL.add)

    nc.sync.dma_start(out=out_ap, in_=outt[:])


_CACHE = {}


def _build(Tval=None):
    if "nc" in _CACHE:
        return _CACHE["nc"]
    nc = bacc.Bacc("TRN2", target_bir_lowering=False, debug=False)
    ins = {}
    for name, (shape, dt) in INPUTS.items():
        ins[name] = nc.dram_tensor(name, list(shape), dt,
                                   kind="ExternalInput").ap()
    out_ap = nc.dram_tensor("out", [P, 32], f32, kind="ExternalOutput").ap()
    with tile.TileContext(nc) as tc:
        with ExitStack() as ctx:
            _body(ctx, tc, ins, out_ap)
    nc.compile()
    _CACHE["nc"] = (nc, ins, out_ap)
    return _CACHE["nc"]


def host_prep(mu_raw, log_alpha, log_beta):
    """O(D^2) parameter transforms in float64."""
    mu = np.log1p(np.exp(np.float64(mu_raw)))
    al = np.log1p(np.exp(np.float64(log_alpha)))
    be = np.log1p(np.exp(np.float64(log_beta)))
    ab = al * be
    return mu, al, be, ab


def make_in_maps(time_points, event_types, mu_raw, log_alpha, log_beta, T):
    Tval = float(np.asarray(T))
    tp = np.asarray(time_points, dtype=np.float32)
    et = np.asarray(event_types).astype(np.int64)
    mu, al, be, ab = host_prep(np.asarray(mu_raw), np.asarray(log_alpha),
                               np.asarray(log_beta))
    ab32 = ab.astype(np.float32)
    be32 = be.astype(np.float32)

    # batch-independent pieces
    msc_c = np.zeros((P, 388), dtype=ml_dtypes.bfloat16)
    msc_c[:, 160:288] = np.triu(np.ones((P, P), dtype=np.float32))
    msc_c[0:RM, 288:388] = np.eye(RM, dtype=np.float32)

    # block-diag gather tables: block c rows 10c:10c+10, cols 20c:20c+20.
    # gathered[j, 20c+t] = sum_d [e_j == d] * tabs[10c+d, 20c+t], so row d
    # holds (be.T[d, :] | ln(al.T)[d, :]) = (beta[:, d] | ln alpha[:, d]).
    tabs = np.zeros((80, 160), dtype=np.float64)
    lnalT = np.log(al.T)
    for c in range(8):
        tabs[10 * c:10 * (c + 1), 20 * c:20 * c + 10] = be.T
        tabs[10 * c:10 * (c + 1), 20 * c + 10:20 * c + 20] = lnalT
    tabs_bf = tabs.astype(ml_dtypes.bfloat16)

    in_maps = []
    for b in range(B):
        t2d = tp[b].reshape(KC, P).T                  # [128, 16]
        e2d = et[b].reshape(KC, P).T                  # [128, 16]
        ts = tp[b, ::P]                               # [16]
        dtb = np.zeros(KC, dtype=np.float64)
        dtb[:-1] = (ts[1:] - ts[:-1]).astype(np.float64)

        a32 = np.zeros((P, 148), dtype=np.float32)
        a32[:, 0:16] = t2d - ts[None, :]
        a32[:, 16:32] = t2d - np.float32(Tval)
        a32[:, 32:132] = np.broadcast_to(
            be32.reshape(-1), (P, RM))
        a32[0:RM, 132:148] = np.exp(
            -be.reshape(-1)[:, None] * dtb[None, :]).astype(np.float32)

        # maskab[j, c, r, m] = ab[r, m] * [e2d[j,c] == r]
        maskab = np.zeros((P, KC, D, D), dtype=np.float32)
        # gather rows of ab by event type
        maskab_rows = ab32[e2d.reshape(-1)]           # [(P*KC), D] = ab[e, m]
        jj = np.arange(P).repeat(KC)
        cc = np.tile(np.arange(KC), P)
        maskab[jj, cc, e2d.reshape(-1)] = maskab_rows
        mka = maskab.reshape(P, KC * RM).astype(ml_dtypes.bfloat16)

        msc = msc_c.copy()
        ohmat = (e2d[:, :, None] ==
                 np.arange(D)[None, None, :]).astype(np.float32)
        msc[:, 0:160] = ohmat.reshape(P, 160)

        gro = np.zeros((80, 416), dtype=ml_dtypes.bfloat16)
        for h in range(2):
            for c in range(8):
                ch = 8 * h + c
                oh_ch = (et[b][128 * ch:128 * (ch + 1)][None, :] ==
                         np.arange(D)[:, None]).astype(np.float32)
                gro[10 * c:10 * (c + 1), 128 * h:128 * (h + 1)] = oh_ch
        gro[:, 256:416] = tabs_bf

        in_maps.append({"a32": a32, "mka": mka, "msc": msc, "gro": gro})
    return in_maps, Tval


def kernel(time_points, event_types, mu_raw, log_alpha, log_beta, T):
    in_maps, Tval = make_in_maps(time_points, event_types, mu_raw,
                                 log_alpha, log_beta, T)
    nc, _, _ = _build()
    res = run_bass_kernel_spmd(nc, in_maps, list(range(B))).results

    et = np.asarray(event_types).astype(np.int64)
    mu, al, be, ab = host_prep(np.asarray(mu_raw), np.asarray(log_alpha),
                               np.asarray(log_beta))
    musub = mu - np.diag(ab)          # compensates the j==i self pair
    asum = al.sum(axis=0)
    out = np.zeros(B, dtype=np.float64)
    for b in range(B):
        r = np.asarray(res[b]["out"], dtype=np.float64)   # [128, 32]
        e2d = et[b].reshape(KC, P).T
        lam = r[:, 0:16] + musub[e2d]
        pos = np.log(lam).sum()
        neg = asum[et[b]].sum() - r[:, 16:32].sum()
        out[b] = pos - Tval * mu.sum() - neg
    return out.astype(np.float32)


# revision 22
# speedup vs baseline: 1.0363x; 1.0363x over previous
"""Trainium2 Bass kernel for the exp-kernel multivariate Hawkes process
log-likelihood (B=8, N=2048, D=10).

v2 strategy (rewrite of the chunked O(N*D^2) baseline)
------------------------------------------------------
Data-parallel over batch: core b computes batch row b. Chunks of C=128
events; per chunk the pairwise interaction is a PE prefix-matmul over
(r,m) type pairs; inter-chunk state S is an affine scan chained PER
GROUP of 4 chunks so the whole S machinery pipelines with the W build
instead of serializing at the end.

Key deltas vs v1:
- maskab host tensor (ab[r,m]*[e_i==r], bf16) folds the receiver mask
  into Q = expU*maskab, eliminating the PSUM-masking pass.
- Per-group scan with `initial` chaining + diag-placement inject
  (K=100 matmul) replaces transpose/selmask/K=16 injects.
- Negative part gathers (bT | ln aT) rows via 2 block-diagonal K=80
  matmuls and fuses the alpha weight into the exp argument.
- Device output is raw (lamr | negsub) [128, 32]; the host does the
  final log/sum reduction in fp64 (O(N) numpy), so no Ln table load
  and no device-side final reduce.
- Inputs consolidated into 4 packed DRAM tensors DMA'd on parallel
  queues immediately at kernel start.
"""
import numpy as np
from contextlib import ExitStack

import ml_dtypes
import concourse.bass as bass
import concourse.mybir as mybir
import concourse.tile as tile
from concourse import bacc
from concourse.bass_utils import run_bass_kernel_spmd

f32 = mybir.dt.float32
bf16 = mybir.dt.bfloat16
AL = mybir.AluOpType
AF = mybir.ActivationFunctionType
AX = mybir.AxisListType

P = 128          # partitions == chunk size
KC = 16          # number of chunks
D = 10           # event types
RM = D * D       # (receiver, trigger) pairs
N = P * KC       # 2048 events per batch row
B = 8            # batch == cores
NG = 4           # chunk groups (4 chunks per PSUM bank)
GC = KC // NG    # chunks per group

# packed DRAM inputs: name -> (shape, dtype)
INPUTS = {
    # trel(16) tau2(16) bflat(100) decayT(16, partitions 0:100)
    "a32": ((P, 148), f32),
    # maskab (c,r,m) flat = 1600
    "mka": ((P, KC * RM), bf16),
    # ohM (c,m) 0:160 | triu 160:288 | diagsel 288:388 (partitions 0:100)
    "msc": ((P, 388), bf16),
    # grows lhsT halves 0:128, 128:256 | block-diag tabs 256:416
    "gro": ((80, 416), bf16),
}


def _body(ctx: ExitStack, tc, ins, out_ap):
    nc = tc.nc
    cpool = ctx.enter_context(tc.tile_pool(name="cpool", bufs=1))
    wpool = ctx.enter_context(tc.tile_pool(name="wpool", bufs=1))
    pp = ctx.enter_context(tc.tile_pool(name="pp", bufs=1, space="PSUM"))
    ps = ctx.enter_context(tc.tile_pool(name="ps", bufs=1, space="PSUM"))

    # ---- input DMAs, all issued first on parallel queues ----
    a32 = cpool.tile([P, 148], f32, tag="a32")
    nc.sync.dma_start(out=a32[:], in_=ins["a32"])
    mka = cpool.tile([P, KC, RM], bf16, tag="mka")
    nc.scalar.dma_start(out=mka[:, 0:KC // 2], in_=ins["mka"][:, 0:800])
    nc.gpsimd.dma_start(out=mka[:, KC // 2:], in_=ins["mka"][:, 800:1600])
    msc = cpool.tile([P, 388], bf16, tag="msc")
    nc.scalar.dma_start(out=msc[:], in_=ins["msc"])
    gro = cpool.tile([80, 416], bf16, tag="gro")
    nc.gpsimd.dma_start(out=gro[:], in_=ins["gro"])

    trel = a32[:, 0:16]
    tau2 = a32[:, 16:32]
    bflat = a32[:, 32:132]
    decayT = a32[0:RM, 132:148]
    ohM = msc[:, 0:160].rearrange("p (c m) -> p c m", c=KC)
    triu = msc[:, 160:288]
    diagsel = msc[0:RM, 288:388]

    # ---- constants ----
    ones_col_bf = cpool.tile([P, 1], bf16, tag="ones_col_bf")
    nc.vector.memset(ones_col_bf[:], 1.0)
    ones100 = cpool.tile([RM, P], bf16, tag="ones100")
    nc.vector.memset(ones100[:], 1.0)
    # SCOLbuf[:, k] = S_k (col 0 = S_0 = 0); scans chain through it.
    # bf16: scan state stays fp32 internally, only stores downcast.
    SCOL = cpool.tile([RM, KC + 1], bf16, tag="SCOL")
    nc.vector.memset(SCOL[:, 0:1], 0.0)

    # ---- big per-event tiles ----
    argW = wpool.tile([P, KC, D, D], f32, tag="argW")
    expW = wpool.tile([P, KC, D, D], bf16, tag="expW")
    expU = wpool.tile([P, KC, D, D], bf16, tag="expU")
    W = wpool.tile([P, KC, D, D], bf16, tag="W")
    Q = wpool.tile([P, KC, RM], bf16, tag="Q")
    G2 = wpool.tile([P, KC, RM], bf16, tag="G2")
    rhsd = wpool.tile([RM, KC, RM], bf16, tag="rhsd")
    outt = wpool.tile([P, 32], f32, tag="outt")

    # asymmetric chunk groups: big early groups pipeline, tiny last group
    # keeps the end-of-kernel serial chain (scan->rhsd->inject->G2->lamr)
    # short
    GROUPS = [(0, 5), (5, 10), (10, 15), (15, 16)]
    Pg = [pp.tile([P, hi - lo, D, D], f32, tag=f"Pg{g}", name=f"Pg{g}")
          for g, (lo, hi) in enumerate(GROUPS)]
    wsum = ps.tile([RM, KC], f32, tag="wsum")

    grows = ps.tile([P, 2, 8, 20], f32, tag="grows")
    grosb = wpool.tile([P, 2, 8, 20], f32, tag="grosb")
    narg = wpool.tile([P, 2, 8, D], f32, tag="narg")
    nexp = wpool.tile([P, 2, 8, D], bf16, tag="nexp")
    NGR = len(GROUPS)

    # Per-group interleaved issue keeps tile-granular dependencies tight
    # (scan_g only waits its own group's wsumc matmuls). Only the G2/lamr
    # tail is deferred to a final phase so those long-stalling ops don't
    # clog the 4-deep DVE wait queue ahead of later groups' argW/Q.
    for h in range(2):
        nc.tensor.matmul(grows[:, h].rearrange("p c t -> p (c t)"),
                         gro[:, 128 * h:128 * (h + 1)],
                         gro[:, 256:416], start=True, stop=True)

    for g, (lo, hi) in enumerate(GROUPS):
        gs, gc = slice(lo, hi), hi - lo
        nc.vector.tensor_tensor(
            out=argW[:, gs],
            in0=trel[:, gs].unsqueeze(2).unsqueeze(3)
                .broadcast_to([P, gc, D, D]),
            in1=bflat.rearrange("p (r m) -> p r m", r=D)
                .unsqueeze(1).broadcast_to([P, gc, D, D]),
            op=AL.mult)
        nc.scalar.activation(expW[:, gs], argW[:, gs], AF.Exp)
        nc.scalar.activation(expU[:, gs], argW[:, gs], AF.Exp, scale=-1.0)
        nc.gpsimd.tensor_tensor(
            out=W[:, gs], in0=expW[:, gs],
            in1=ohM[:, gs].unsqueeze(2).broadcast_to([P, gc, D, D]),
            op=AL.mult)
        nc.vector.tensor_tensor(
            out=Q[:, gs],
            in0=expU[:, gs].rearrange("p c r m -> p c (r m)"),
            in1=mka[:, gs], op=AL.mult)
        for k in range(lo, hi):
            nc.tensor.matmul(wsum[:, k:k + 1],
                             W[:, k].rearrange("p r m -> p (r m)"),
                             ones_col_bf[:], start=True, stop=True)
        nc.tensor.matmul(Pg[g][:], triu,
                         W[:, gs].rearrange("p c r m -> p (c r m)"),
                         start=True, stop=False)
        nc.vector.tensor_tensor_scan(
            SCOL[:, lo + 1:hi + 1], wsum[:, gs],
            decayT[:, gs], initial=SCOL[:, lo:lo + 1],
            op0=AL.add, op1=AL.mult)
        rhsd_eng = nc.vector if g == NGR - 1 else nc.gpsimd
        rhsd_eng.tensor_tensor(
            out=rhsd[:, gs],
            in0=SCOL[:, gs].unsqueeze(2).broadcast_to([RM, gc, RM]),
            in1=diagsel.unsqueeze(1).broadcast_to([RM, gc, RM]),
            op=AL.mult)
        nc.tensor.matmul(Pg[g][:], ones100[:],
                         rhsd[:, gs].rearrange("k c rm -> k (c rm)"),
                         start=False, stop=True)

    # -- negative (integral) part: Pool + scalar, reduces queued on DVE
    # before the G2 tail so they run in the inject shadow --
    for h in range(2):
        nc.scalar.activation(grosb[:, h], grows[:, h], AF.Copy)
        nc.gpsimd.tensor_tensor(
            out=narg[:, h], in0=grosb[:, h, :, 0:10],
            in1=tau2[:, 8 * h:8 * (h + 1)].unsqueeze(2)
                .broadcast_to([P, 8, D]),
            op=AL.mult)
        nc.gpsimd.tensor_tensor(
            out=narg[:, h], in0=narg[:, h], in1=grosb[:, h, :, 10:20],
            op=AL.add)
        nc.scalar.activation(nexp[:, h], narg[:, h], AF.Exp)
        nc.vector.tensor_reduce(
            out=outt[:, 16 + 8 * h:16 + 8 * (h + 1)], in_=nexp[:, h],
            axis=AX.X, op=AL.add)

    # -- phase 4: lam tail --
    for g, (lo, hi) in enumerate(GROUPS):
        gs = slice(lo, hi)
        nc.vector.tensor_tensor(
            out=G2[:, gs],
            in0=Pg[g][:].rearrange("p c r m -> p c (r m)"),
            in1=Q[:, gs], op=AL.mult)
        nc.vector.tensor_reduce(
            out=outt[:, lo:hi], in_=G2[:, gs],
            axis=AX.X, op=AL.add)

    nc.sync.dma_start(out=out_ap, in_=outt[:])


_CACHE = {}


def _build(Tval=None):
    if "nc" in _CACHE:
        return _CACHE["nc"]
    nc = bacc.Bacc("TRN2", target_bir_lowering=False, debug=False)
    ins = {}
    for name, (shape, dt) in INPUTS.items():
        ins[name] = nc.dram_tensor(name, list(shape), dt,
                                   kind="ExternalInput").ap()
    out_ap = nc.dram_tensor("out", [P, 32], f32, kind="ExternalOutput").ap()
    with tile.TileContext(nc) as tc:
        with ExitStack() as ctx:
            _body(ctx, tc, ins, out_ap)
    nc.compile()
    _CACHE["nc"] = (nc, ins, out_ap)
    return _CACHE["nc"]


def host_prep(mu_raw, log_alpha, log_beta):
    """O(D^2) parameter transforms in float64."""
    mu = np.log1p(np.exp(np.float64(mu_raw)))
    al = np.log1p(np.exp(np.float64(log_alpha)))
    be = np.log1p(np.exp(np.float64(log_beta)))
    ab = al * be
    return mu, al, be, ab


def make_in_maps(time_points, event_types, mu_raw, log_alpha, log_beta, T):
    Tval = float(np.asarray(T))
    tp = np.asarray(time_points, dtype=np.float32)
    et = np.asarray(event_types).astype(np.int64)
    mu, al, be, ab = host_prep(np.asarray(mu_raw), np.asarray(log_alpha),
                               np.asarray(log_beta))
    ab32 = ab.astype(np.float32)
    be32 = be.astype(np.float32)

    # batch-independent pieces
    msc_c = np.zeros((P, 388), dtype=ml_dtypes.bfloat16)
    msc_c[:, 160:288] = np.triu(np.ones((P, P), dtype=np.float32))
    msc_c[0:RM, 288:388] = np.eye(RM, dtype=np.float32)

    # block-diag gather tables: block c rows 10c:10c+10, cols 20c:20c+20.
    # gathered[j, 20c+t] = sum_d [e_j == d] * tabs[10c+d, 20c+t], so row d
    # holds (be.T[d, :] | ln(al.T)[d, :]) = (beta[:, d] | ln alpha[:, d]).
    tabs = np.zeros((80, 160), dtype=np.float64)
    lnalT = np.log(al.T)
    for c in range(8):
        tabs[10 * c:10 * (c + 1), 20 * c:20 * c + 10] = be.T
        tabs[10 * c:10 * (c + 1), 20 * c + 10:20 * c + 20] = lnalT
    tabs_bf = tabs.astype(ml_dtypes.bfloat16)

    in_maps = []
    for b in range(B):
        t2d = tp[b].reshape(KC, P).T                  # [128, 16]
        e2d = et[b].reshape(KC, P).T                  # [128, 16]
        ts = tp[b, ::P]                               # [16]
        dtb = np.zeros(KC, dtype=np.float64)
        dtb[:-1] = (ts[1:] - ts[:-1]).astype(np.float64)

        a32 = np.zeros((P, 148), dtype=np.float32)
        a32[:, 0:16] = t2d - ts[None, :]
        a32[:, 16:32] = t2d - np.float32(Tval)
        a32[:, 32:132] = np.broadcast_to(
            be32.reshape(-1), (P, RM))
        a32[0:RM, 132:148] = np.exp(
            -be.reshape(-1)[:, None] * dtb[None, :]).astype(np.float32)

        # maskab[j, c, r, m] = ab[r, m] * [e2d[j,c] == r]
        maskab = np.zeros((P, KC, D, D), dtype=np.float32)
        # gather rows of ab by event type
        maskab_rows = ab32[e2d.reshape(-1)]           # [(P*KC), D] = ab[e, m]
        jj = np.arange(P).repeat(KC)
        cc = np.tile(np.arange(KC), P)
        maskab[jj, cc, e2d.reshape(-1)] = maskab_rows
        mka = maskab.reshape(P, KC * RM).astype(ml_dtypes.bfloat16)

        msc = msc_c.copy()
        ohmat = (e2d[:, :, None] ==
                 np.arange(D)[None, None, :]).astype(np.float32)
        msc[:, 0:160] = ohmat.reshape(P, 160)

        gro = np.zeros((80, 416), dtype=ml_dtypes.bfloat16)
        for h in range(2):
            for c in range(8):
                ch = 8 * h + c
                oh_ch = (et[b][128 * ch:128 * (ch + 1)][None, :] ==
                         np.arange(D)[:, None]).astype(np.float32)
                gro[10 * c:10 * (c + 1), 128 * h:128 * (h + 1)] = oh_ch
        gro[:, 256:416] = tabs_bf

        in_maps.append({"a32": a32, "mka": mka, "msc": msc, "gro": gro})
    return in_maps, Tval


def kernel(time_points, event_types, mu_raw, log_alpha, log_beta, T):
    in_maps, Tval = make_in_maps(time_points, event_types, mu_raw,
                                 log_alpha, log_beta, T)
    nc, _, _ = _build()
    res = run_bass_kernel_spmd(nc, in_maps, list(range(B))).results

    et = np.asarray(event_types).astype(np.int64)
    mu, al, be, ab = host_prep(np.asarray(mu_raw), np.asarray(log_alpha),
                               np.asarray(log_beta))
    musub = mu - np.diag(ab)          # compensates the j==i self pair
    asum = al.sum(axis=0)
    out = np.zeros(B, dtype=np.float64)
    for b in range(B):
        r = np.asarray(res[b]["out"], dtype=np.float64)   # [128, 32]
        e2d = et[b].reshape(KC, P).T
        lam = r[:, 0:16] + musub[e2d]
        pos = np.log(lam).sum()
        neg = asum[et[b]].sum() - r[:, 16:32].sum()
        out[b] = pos - Tval * mu.sum() - neg
    return out.astype(np.float32)
